# revision 1
# baseline (speedup 1.0000x reference)
"""Gaussian point-cloud rasterization on 8 Trainium2 NeuronCores (Bass/Tile).

Strategy (pixel-sharded, points replicated):
 - 8 cores x 32 image rows each; per core 16 tiles of 512 pixels.
 - Points (N=256) live on partitions in 2 blocks of 128.
 - Depth sort + cumsum-compositing is reformulated as C = S @ a with a
   host-built 0/1 "sorts-before" matrix S (no device sort needed); the
   (1 - acc_before) term uses (I - S) @ a so signs work out with the
   fused DVE ops available.
 - Gaussian log-density is a K=6 matmul of per-point coefficients against
   the per-pixel basis [1, px^2, py^2, px*py, px, py]; opacity and the
   det-normalizer are folded into the constant term, so alpha needs only
   exp + two fused select ops.
 - SH color is a K=16 matmul; sigmoid(x) = 0.5*tanh(x/2)+0.5 so that exp
   and tanh share one ACT table set (no ~2.7us table switches).
 - The 0.5 scale/offset of the tanh trick folds into the PE reduction
   weights (0.5-valued lhsT vectors + one extra accumulating matmul).
"""
import sys
import numpy as np

sys.path.insert(0, "/opt/trn_rl_repo")

N = 256
H = W = 256
NCORES = 8
ROWS = H // NCORES          # 32
PCORE = ROWS * W            # 8192
TILE = 512
NT = PCORE // TILE          # 16
CENTER = 128.0

LN_CLAMP = float(np.float32(np.log(0.99)))        # alpha clamp in logit space
LN_SKIP = float(np.float32(np.log(1.0 / 255.0)))  # alpha skip threshold in logit space
ACC_BREAK = 0.9999

_C0 = 0.28209479177387814
_C1 = 0.4886025119029199
_C2 = (1.0925484305920792, -1.0925484305920792, 0.31539156525252005,
       -1.0925484305920792, 0.5462742152960396)
_C3 = (-0.5900435899266435, 2.890611442640554, -0.4570457994644658, 0.3731763325901154,
       -0.4570457994644658, 1.445305721320277, -0.5900435899266435)

# how many of the 6 per-tile (wgt * tanh) products run on DVE vs GPSIMD
_PROD_ON_VECTOR = (0, 1, 2, 3, 4, 5)


def _host_preprocess(pointcloud, feats, K, T):
    f32 = np.float32
    pc = np.asarray(pointcloud, f32)
    feats = np.asarray(feats, f32)
    K = np.asarray(K, f32)
    T = np.asarray(T, f32)
    R, t = T[:3, :3], T[:3, 3]
    p_cam = pc @ R.T + t
    zc = p_cam[:, 2]
    proj = p_cam @ K.T
    uv = proj[:, :2] / np.clip(zc, 1e-6, None)[:, None]
    in_cam = ((zc > 0.8) & (zc < 1000.0) & (uv[:, 0] >= 0) & (uv[:, 0] < W)
              & (uv[:, 1] >= 0) & (uv[:, 1] < H))
    zs = np.where(in_cam, zc, f32(1e10)).astype(f32)
    idx = np.arange(N)
    # S[i,j] = 1 iff point j composites at-or-before point i under a stable
    # argsort of zs (ties only matter for culled points, which have a = 0)
    S = ((zs[None, :] < zs[:, None])
         | ((zs[None, :] == zs[:, None]) & (idx[None, :] <= idx[:, None]))).astype(f32)
    Sneg = (np.eye(N, dtype=f32) - S).astype(f32)   # (I-S)@a = a - C = -acc_before

    q = feats[:, :4]
    q = q / np.linalg.norm(q, axis=-1, keepdims=True).astype(f32)
    x, y, z, w = q[:, 0], q[:, 1], q[:, 2], q[:, 3]
    Rq = np.stack([
        1 - 2 * (y * y + z * z), 2 * (x * y - z * w), 2 * (x * z + y * w),
        2 * (x * y + z * w), 1 - 2 * (x * x + z * z), 2 * (y * z - x * w),
        2 * (x * z - y * w), 2 * (y * z + x * w), 1 - 2 * (x * x + y * y)],
        axis=-1).reshape(-1, 3, 3).astype(f32)
    s = np.exp(feats[:, 4:7])
    M = Rq * s[:, None, :]
    Sigma = M @ M.transpose(0, 2, 1)
    fx, fy = K[0, 0], K[1, 1]
    zero = np.zeros_like(zc)
    J = np.stack([
        np.stack([fx / zc, zero, -fx * p_cam[:, 0] / (zc * zc)], -1),
        np.stack([zero, fy / zc, -fy * p_cam[:, 1] / (zc * zc)], -1)], axis=-2)
    JW = J @ R
    cov = JW @ Sigma @ JW.transpose(0, 2, 1)
    det = np.maximum(cov[:, 0, 0] * cov[:, 1, 1] - cov[:, 0, 1] * cov[:, 1, 0], 1e-12)
    ia, ib, ic = cov[:, 1, 1] / det, -cov[:, 0, 1] / det, cov[:, 0, 0] / det

    sig_op = 1.0 / (1.0 + np.exp(-feats[:, 7].astype(np.float64)))
    lg = np.log(sig_op) - np.log(2 * np.pi) - 0.5 * np.log(det.astype(np.float64))

    ia64, ib64, ic64 = ia.astype(np.float64), ib.astype(np.float64), ic.astype(np.float64)
    ux = np.clip(uv[:, 0].astype(np.float64) - CENTER, -1e4, 1e4)
    uy = np.clip(uv[:, 1].astype(np.float64) - CENTER, -1e4, 1e4)
    k0 = ia64 * ux * ux + ic64 * uy * uy + 2 * ib64 * ux * uy
    kx = ia64 * ux + ib64 * uy
    ky = ic64 * uy + ib64 * ux
    A = np.stack([lg - 0.5 * k0, -0.5 * ia64, -0.5 * ic64, -ib64, kx, ky]).astype(f32)
    A[0, ~in_cam] = f32(-1e20)

    coeffs = feats[:, 8:56].reshape(N, 3, 16)
    coefft = np.ascontiguousarray(coeffs.transpose(2, 1, 0).reshape(16, 3 * N)).astype(f32)

    wv = np.arange(W, dtype=np.float64) + 0.5 - CENTER
    hv = np.arange(H, dtype=np.float64) + 0.5 - CENTER
    pxg, pyg = np.meshgrid(wv, hv)
    px = pxg.reshape(-1)
    py = pyg.reshape(-1)
    bpix = np.stack([np.ones_like(px), px * px, py * py, px * py, px, py]).astype(f32)

    Kinv = np.linalg.inv(K.astype(np.float64))
    ug, vg = np.meshgrid(np.arange(W, dtype=np.float64), np.arange(H, dtype=np.float64))
    pix = np.stack([ug, vg, np.ones_like(ug)], axis=-1)
    d = (pix @ Kinv.T) @ R.astype(np.float64)
    d = d / np.linalg.norm(d, axis=-1, keepdims=True)
    dx_, dy_, dz_ = d[..., 0], d[..., 1], d[..., 2]
    xx, yy, zz = dx_ * dx_, dy_ * dy_, dz_ * dz_
    shb = np.stack([
        np.full_like(dx_, _C0),
        -_C1 * dy_, _C1 * dz_, -_C1 * dx_,
        _C2[0] * dx_ * dy_, _C2[1] * dy_ * dz_, _C2[2] * (2 * zz - xx - yy),
        _C2[3] * dx_ * dz_, _C2[4] * (xx - yy),
        _C3[0] * dy_ * (3 * xx - yy), _C3[1] * dx_ * dy_ * dz_,
        _C3[2] * dy_ * (4 * zz - xx - yy),
        _C3[3] * dz_ * (2 * zz - 3 * xx - 3 * yy), _C3[4] * dx_ * (4 * zz - xx - yy),
        _C3[5] * dz_ * (xx - yy), _C3[6] * dx_ * (xx - 3 * yy)],
        axis=0).reshape(16, H * W).astype(f32)

    stp = np.zeros((128, 4, 128), f32)
    stn = np.zeros((128, 4, 128), f32)
    for bi in range(2):
        for bj in range(2):
            stp[:, bi * 2 + bj, :] = S[bi * 128:(bi + 1) * 128, bj * 128:(bj + 1) * 128].T
            stn[:, bi * 2 + bj, :] = Sneg[bi * 128:(bi + 1) * 128, bj * 128:(bj + 1) * 128].T

    # reduction weights: slot 4g+0 sums 0.5*wgt into img rows 3g..3g+2,
    # slot 4g+1+c sums 0.5*prod into img row 3g+c (rows of a [12,TILE] psum bank
    # holding 4 consecutive pixel tiles' rgb rows)
    zh = np.zeros((128, 16, 12), f32)
    for g in range(4):
        zh[:, 4 * g + 0, 3 * g:3 * g + 3] = 0.5
        for c in range(3):
            zh[:, 4 * g + 1 + c, 3 * g + c] = 0.5
    return dict(A=A, stp=stp, stn=stn, coefft=coefft, bpix=bpix, shb=shb, zh=zh)


_NC_CACHE = {}


def _build_nc(repeats=1):
    key = ("nc", repeats)
    if key in _NC_CACHE:
        return _NC_CACHE[key]
    from contextlib import ExitStack
    import concourse.tile as tile
    from concourse import bacc, mybir

    f32 = mybir.dt.float32
    op = mybir.AluOpType
    act = mybir.ActivationFunctionType

    nc = bacc.Bacc(None, target_bir_lowering=False, debug=False)
    bpix_d = nc.dram_tensor("bpix", [6, PCORE], f32, kind="ExternalInput")
    shb_d = nc.dram_tensor("shb", [16, PCORE], f32, kind="ExternalInput")
    apr_d = nc.dram_tensor("aprime", [6, N], f32, kind="ExternalInput")
    stp_d = nc.dram_tensor("stpos", [128, 4, 128], f32, kind="ExternalInput")
    stn_d = nc.dram_tensor("stneg", [128, 4, 128], f32, kind="ExternalInput")
    cft_d = nc.dram_tensor("coefft", [16, 3 * N], f32, kind="ExternalInput")
    zh_d = nc.dram_tensor("zh", [128, 16, 12], f32, kind="ExternalInput")
    # [q, 3g+c, n]: channel c of pixel tile ti = 4q+g
    img_d = nc.dram_tensor("img", [NT // 4, 12, TILE], f32, kind="ExternalOutput")

    with tile.TileContext(nc) as tc, ExitStack() as ctx:
        const = ctx.enter_context(tc.tile_pool(name="const", bufs=1))
        work = ctx.enter_context(tc.tile_pool(name="work", bufs=3))
        keep = ctx.enter_context(tc.tile_pool(name="keep", bufs=4))
        ps_q = ctx.enter_context(tc.tile_pool(name="ps_q", bufs=2, space="PSUM"))
        ps_c = ctx.enter_context(tc.tile_pool(name="ps_c", bufs=1, space="PSUM"))
        ps_col = ctx.enter_context(tc.tile_pool(name="ps_col", bufs=2, space="PSUM"))
        ps_img = ctx.enter_context(tc.tile_pool(name="ps_img", bufs=2, space="PSUM"))

        bpix = const.tile([6, PCORE], f32)
        nc.sync.dma_start(out=bpix[:], in_=bpix_d[:])
        shb = const.tile([16, PCORE], f32)
        nc.sync.dma_start(out=shb[:], in_=shb_d[:])
        apr = const.tile([6, N], f32)
        nc.sync.dma_start(out=apr[:], in_=apr_d[:])
        stp = const.tile([128, 4, 128], f32)
        nc.sync.dma_start(out=stp[:], in_=stp_d[:])
        stn = const.tile([128, 4, 128], f32)
        nc.sync.dma_start(out=stn[:], in_=stn_d[:])
        cft = const.tile([16, 3 * N], f32)
        nc.sync.dma_start(out=cft[:], in_=cft_d[:])
        zh = const.tile([128, 16, 12], f32)
        nc.sync.dma_start(out=zh[:], in_=zh_d[:])

        img = None
        for ti_rep in range(NT * repeats):
            ti = ti_rep % NT
            sl = slice(ti * TILE, (ti + 1) * TILE)
            g = ti % 4
            if g == 0:
                img = ps_img.tile([12, TILE], f32, tag="img")
            quads, a_s = [], []
            for b in range(2):
                quad = ps_q.tile([128, TILE], f32, tag="quad")
                nc.tensor.matmul(quad[:], apr[:, b * 128:(b + 1) * 128], bpix[:, sl],
                                 start=True, stop=True)
                t_ = work.tile([128, TILE], f32, tag="t_")
                nc.vector.tensor_scalar(out=t_[:], in0=quad[:], scalar1=LN_CLAMP,
                                        scalar2=None, op0=op.min)
                ex = work.tile([128, TILE], f32, tag="ex")
                nc.scalar.activation(ex[:], t_[:], act.Exp)
                av = keep.tile([128, TILE], f32, tag="av")
                nc.vector.scalar_tensor_tensor(out=av[:], in0=quad[:], scalar=LN_SKIP,
                                               in1=ex[:], op0=op.is_ge, op1=op.mult)
                quads.append(quad)
                a_s.append(av)
            wgts = []
            for b in range(2):
                Cp = ps_c.tile([128, TILE], f32, tag="Cp")
                Cn = ps_c.tile([128, TILE], f32, tag="Cn")
                for bj in range(2):
                    nc.tensor.matmul(Cp[:], stp[:, b * 2 + bj, :], a_s[bj][:],
                                     start=(bj == 0), stop=(bj == 1))
                    nc.tensor.matmul(Cn[:], stn[:, b * 2 + bj, :], a_s[bj][:],
                                     start=(bj == 0), stop=(bj == 1))
                w1 = work.tile([128, TILE], f32, tag="w1")
                nc.vector.scalar_tensor_tensor(out=w1[:], in0=Cn[:], scalar=-1.0,
                                               in1=a_s[b][:], op0=op.subtract, op1=op.mult)
                wgt = keep.tile([128, TILE], f32, tag="wgt")
                nc.vector.scalar_tensor_tensor(out=wgt[:], in0=Cp[:], scalar=ACC_BREAK,
                                               in1=w1[:], op0=op.is_le, op1=op.mult)
                wgts.append(wgt)
            for b in range(2):
                nc.tensor.matmul(img[:], zh[:, 4 * g + 0, :], wgts[b][:],
                                 start=(g == 0 and b == 0), stop=False)
            k = 0
            for c in range(3):
                for b in range(2):
                    col = ps_col.tile([128, TILE], f32, tag="col")
                    nc.tensor.matmul(col[:], cft[:, c * N + b * 128:c * N + (b + 1) * 128],
                                     shb[:, sl], start=True, stop=True)
                    th = work.tile([128, TILE], f32, tag="th")
                    nc.scalar.activation(th[:], col[:], act.Tanh, scale=0.5)
                    prod = work.tile([128, TILE], f32, tag="prod")
                    eng = nc.vector if (k in _PROD_ON_VECTOR) else nc.gpsimd
                    eng.tensor_mul(prod[:], wgts[b][:], th[:])
                    nc.tensor.matmul(img[:], zh[:, 4 * g + 1 + c, :], prod[:],
                                     start=False, stop=(g == 3 and c == 2 and b == 1))
                    k += 1
            if g == 3:
                sbimg = work.tile([12, TILE], f32, tag="sbimg")
                nc.scalar.copy(sbimg[:], img[:])
                nc.sync.dma_start(out=img_d[ti // 4], in_=sbimg[:])
    nc.compile()
    _NC_CACHE[key] = nc
    return nc


def _run(inputs, trace=False, repeats=1):
    from concourse.bass_utils import run_bass_kernel_spmd

    pre = _host_preprocess(inputs["pointcloud"], inputs["pointcloud_features"],
                           inputs["camera_intrinsics"], inputs["T_camera_pointcloud"])
    nc = _build_nc(repeats)
    in_maps = []
    for core in range(NCORES):
        p0 = core * PCORE
        in_maps.append({
            "bpix": np.ascontiguousarray(pre["bpix"][:, p0:p0 + PCORE]),
            "shb": np.ascontiguousarray(pre["shb"][:, p0:p0 + PCORE]),
            "aprime": pre["A"],
            "stpos": pre["stp"],
            "stneg": pre["stn"],
            "coefft": pre["coefft"],
            "zh": pre["zh"],
        })
    bkr = run_bass_kernel_spmd(nc, in_maps, list(range(NCORES)), trace=trace)
    out = np.zeros((H, W, 3), np.float32)
    for core in range(NCORES):
        img = bkr.results[core]["img"]  # [NT//4, 12, TILE]
        flat = np.transpose(img.reshape(NT // 4, 4, 3, TILE), (2, 0, 1, 3)).reshape(3, PCORE)
        out[core * ROWS:(core + 1) * ROWS] = flat.reshape(3, ROWS, W).transpose(1, 2, 0)
    return out, bkr


def kernel(**inputs):
    return _run(inputs)[0]



# revision 4
# speedup vs baseline: 11.3021x; 11.3021x over previous
"""Gaussian point-cloud rasterization on 8 Trainium2 NeuronCores (Bass/Tile).

Strategy (pixel-sharded, points host-culled per core):
 - 8 cores x 32 image rows each; per core 16 tiles of 512 pixels.
 - On host, points are culled per core: only points whose Gaussian support
   (alpha >= 1/255) intersects the core's 32-row band can contribute
   anything (alpha-skip zeroes the rest exactly).  For these inputs only a
   handful survive per band, so each core handles <= 21 points (padded).
 - Points are depth-sorted on host; compositing prefix sums become a
   single lower-triangular matmul (K=21) per pixel tile.
 - One fused matmul per tile computes BOTH the Gaussian log-density
   (quad, via a 6-term pixel basis) and the SH color (16-term basis) for
   all 3 channels: lhsT [22, 128] x basis [22, 512] -> rows 0-62 =
   3x-replicated quad, rows 64-126 = per-channel SH color.
 - sigmoid(x) = 0.5*tanh(x/2)+0.5; the 0.5 scale/offset folds into the
   0.5-valued reduction weights of the final per-tile matmul.
 - All elementwise work runs as a few frame-wide [63, 8192] DVE ops
   (this backend charges per instruction, so wide ops are nearly free).
 - Falls back to the original 256-point replicated kernel if any core's
   culled point count exceeds 21.
"""
import sys
import numpy as np

sys.path.insert(0, "/opt/trn_rl_repo")

N = 256
H = W = 256
NCORES = 8
ROWS = H // NCORES          # 32
PCORE = ROWS * W            # 8192
TILE = 512
NT = PCORE // TILE          # 16
CENTER = 128.0
PHAT = 21                   # fast-path padded points per core (3*21+21 rows <= 128)

ALPHA_SKIP = 1.0 / 255.0
ALPHA_CLAMP = 0.99
ACC_BREAK = 0.9999
LN_CLAMP = float(np.float32(np.log(0.99)))        # alpha clamp in logit space
LN_SKIP = float(np.float32(np.log(1.0 / 255.0)))  # alpha skip threshold in logit space

_C0 = 0.28209479177387814
_C1 = 0.4886025119029199
_C2 = (1.0925484305920792, -1.0925484305920792, 0.31539156525252005,
       -1.0925484305920792, 0.5462742152960396)
_C3 = (-0.5900435899266435, 2.890611442640554, -0.4570457994644658, 0.3731763325901154,
       -0.4570457994644658, 1.445305721320277, -0.5900435899266435)


def _host_geo(pointcloud, feats, K, T):
    """Shared per-point camera/covariance math (f64) + pixel bases."""
    f32 = np.float32
    pc = np.asarray(pointcloud, np.float64)
    feats = np.asarray(feats, np.float64)
    K = np.asarray(K, np.float64)
    T = np.asarray(T, np.float64)
    R, t = T[:3, :3], T[:3, 3]
    p_cam = pc @ R.T + t
    zc = p_cam[:, 2]
    proj = p_cam @ K.T
    uv = proj[:, :2] / np.clip(zc, 1e-6, None)[:, None]
    in_cam = ((zc > 0.8) & (zc < 1000.0) & (uv[:, 0] >= 0) & (uv[:, 0] < W)
              & (uv[:, 1] >= 0) & (uv[:, 1] < H))
    zs = np.where(in_cam, zc, 1e10)

    q = feats[:, :4]
    q = q / np.linalg.norm(q, axis=-1, keepdims=True)
    x, y, z, w = q[:, 0], q[:, 1], q[:, 2], q[:, 3]
    Rq = np.stack([
        1 - 2 * (y * y + z * z), 2 * (x * y - z * w), 2 * (x * z + y * w),
        2 * (x * y + z * w), 1 - 2 * (x * x + z * z), 2 * (y * z - x * w),
        2 * (x * z - y * w), 2 * (y * z + x * w), 1 - 2 * (x * x + y * y)],
        axis=-1).reshape(-1, 3, 3)
    s = np.exp(feats[:, 4:7])
    M = Rq * s[:, None, :]
    Sigma = M @ M.transpose(0, 2, 1)
    fx, fy = K[0, 0], K[1, 1]
    zero = np.zeros_like(zc)
    J = np.stack([
        np.stack([fx / zc, zero, -fx * p_cam[:, 0] / (zc * zc)], -1),
        np.stack([zero, fy / zc, -fy * p_cam[:, 1] / (zc * zc)], -1)], axis=-2)
    JW = J @ R
    cov = JW @ Sigma @ JW.transpose(0, 2, 1)
    det = np.maximum(cov[:, 0, 0] * cov[:, 1, 1] - cov[:, 0, 1] * cov[:, 1, 0], 1e-12)
    ia, ib, ic = cov[:, 1, 1] / det, -cov[:, 0, 1] / det, cov[:, 0, 0] / det

    sig_op = 1.0 / (1.0 + np.exp(-feats[:, 7]))
    lg = np.log(sig_op) - np.log(2 * np.pi) - 0.5 * np.log(det)

    ux = np.clip(uv[:, 0] - CENTER, -1e4, 1e4)
    uy = np.clip(uv[:, 1] - CENTER, -1e4, 1e4)
    k0 = ia * ux * ux + ic * uy * uy + 2 * ib * ux * uy
    kx = ia * ux + ib * uy
    ky = ic * uy + ib * ux
    A = np.stack([lg - 0.5 * k0, -0.5 * ia, -0.5 * ic, -ib, kx, ky]).astype(f32)
    A[0, ~in_cam] = f32(-1e20)

    coeffs = np.asarray(feats[:, 8:56], f32).reshape(N, 3, 16)

    wv = np.arange(W, dtype=np.float64) + 0.5 - CENTER
    hv = np.arange(H, dtype=np.float64) + 0.5 - CENTER
    pxg, pyg = np.meshgrid(wv, hv)
    px = pxg.reshape(-1)
    py = pyg.reshape(-1)
    bpix = np.stack([np.ones_like(px), px * px, py * py, px * py, px, py]).astype(f32)

    Kinv = np.linalg.inv(K)
    ug, vg = np.meshgrid(np.arange(W, dtype=np.float64), np.arange(H, dtype=np.float64))
    pix = np.stack([ug, vg, np.ones_like(ug)], axis=-1)
    d = (pix @ Kinv.T) @ R
    d = d / np.linalg.norm(d, axis=-1, keepdims=True)
    dx_, dy_, dz_ = d[..., 0], d[..., 1], d[..., 2]
    xx, yy, zz = dx_ * dx_, dy_ * dy_, dz_ * dz_
    shb = np.stack([
        np.full_like(dx_, _C0),
        -_C1 * dy_, _C1 * dz_, -_C1 * dx_,
        _C2[0] * dx_ * dy_, _C2[1] * dy_ * dz_, _C2[2] * (2 * zz - xx - yy),
        _C2[3] * dx_ * dz_, _C2[4] * (xx - yy),
        _C3[0] * dy_ * (3 * xx - yy), _C3[1] * dx_ * dy_ * dz_,
        _C3[2] * dy_ * (4 * zz - xx - yy),
        _C3[3] * dz_ * (2 * zz - 3 * xx - 3 * yy), _C3[4] * dx_ * (4 * zz - xx - yy),
        _C3[5] * dz_ * (xx - yy), _C3[6] * dx_ * (xx - 3 * yy)],
        axis=0).reshape(16, H * W).astype(f32)

    return dict(A=A, coeffs=coeffs, uv=uv, zs=zs, in_cam=in_cam, cov=cov, lg=lg,
                bpix=bpix, shb=shb)


def _cull_cores(geo):
    """Per-core culled, depth-sorted point index lists (conservative)."""
    uv, zs, in_cam, cov, lg = geo["uv"], geo["zs"], geo["in_cam"], geo["cov"], geo["lg"]
    # alpha >= (1/255)*0.9 support radius in y (10% threshold margin + 2px)
    r2m = 2.0 * (lg - np.log(0.9 / 255.0))
    alive = in_cam & (r2m > 0)
    ry = np.sqrt(np.maximum(r2m, 0.0) * np.maximum(cov[:, 1, 1], 0.0)) + 2.0
    order = np.argsort(zs, kind="stable")
    rank = np.empty(N, np.int64)
    rank[order] = np.arange(N)
    sels = []
    for c in range(NCORES):
        y0, y1 = c * ROWS + 0.5, c * ROWS + ROWS - 0.5
        sel = np.where(alive & (uv[:, 1] + ry >= y0) & (uv[:, 1] - ry <= y1))[0]
        sel = sel[np.argsort(rank[sel], kind="stable")]
        sels.append(sel)
    return sels


def _host_fast(geo, sels):
    """Per-core tensors for the fused <=21-point kernel."""
    f32 = np.float32
    A, coeffs = geo["A"], geo["coeffs"]
    l3 = np.zeros((PHAT, 64), f32)
    for c in range(3):
        for i in range(PHAT):
            l3[:i + 1, c * PHAT + i] = 1.0
    zh3 = np.zeros((63, 4, 12), f32)
    for g in range(4):
        for c in range(3):
            zh3[c * PHAT:(c + 1) * PHAT, g, 3 * g + c] = 0.5
    in_maps = []
    for core in range(NCORES):
        sel = sels[core]
        p = len(sel)
        coefT = np.zeros((22, 128), f32)
        coefT[0, :63] = f32(-1e20)          # dead quad slots -> a = 0
        for c in range(3):
            coefT[:6, c * PHAT:c * PHAT + p] = A[:, sel]
            coefT[6:22, 64 + c * PHAT:64 + c * PHAT + p] = coeffs[sel, c, :].T
        p0 = core * PCORE
        basis = np.concatenate([geo["bpix"][:, p0:p0 + PCORE],
                                geo["shb"][:, p0:p0 + PCORE]], axis=0)
        in_maps.append({
            "basis": np.ascontiguousarray(basis),
            "coeft": coefT,
            "l3": l3,
            "zh3": zh3,
        })
    return in_maps


_NC_CACHE = {}


def _build_nc_fast(repeats=1):
    key = ("fast", repeats)
    if key in _NC_CACHE:
        return _NC_CACHE[key]
    from contextlib import ExitStack
    import concourse.tile as tile
    from concourse import bacc, mybir

    f32 = mybir.dt.float32
    op = mybir.AluOpType
    act = mybir.ActivationFunctionType

    nc = bacc.Bacc(None, target_bir_lowering=False, debug=False)
    basis_d = nc.dram_tensor("basis", [22, PCORE], f32, kind="ExternalInput")
    coef_d = nc.dram_tensor("coeft", [22, 128], f32, kind="ExternalInput")
    l3_d = nc.dram_tensor("l3", [PHAT, 64], f32, kind="ExternalInput")
    zh3_d = nc.dram_tensor("zh3", [63, 4, 12], f32, kind="ExternalInput")
    img_d = nc.dram_tensor("img", [NT // 4, 12, TILE], f32, kind="ExternalOutput")

    with tile.TileContext(nc) as tc, ExitStack() as ctx:
        const = ctx.enter_context(tc.tile_pool(name="const", bufs=1))
        big = ctx.enter_context(tc.tile_pool(name="big", bufs=1))
        work = ctx.enter_context(tc.tile_pool(name="work", bufs=2))
        pq = ctx.enter_context(tc.tile_pool(name="pq", bufs=2, space="PSUM"))
        pc_ = ctx.enter_context(tc.tile_pool(name="pc", bufs=2, space="PSUM"))
        pi = ctx.enter_context(tc.tile_pool(name="pi", bufs=2, space="PSUM"))

        basis = const.tile([22, PCORE], f32)
        nc.sync.dma_start(out=basis[:], in_=basis_d[:])
        coef = const.tile([22, 128], f32)
        nc.sync.dma_start(out=coef[:], in_=coef_d[:])
        l3 = const.tile([PHAT, 64], f32)
        nc.sync.dma_start(out=l3[:], in_=l3_d[:])
        zh3 = const.tile([63, 4, 12], f32)
        nc.sync.dma_start(out=zh3[:], in_=zh3_d[:])

        av = big.tile([63, PCORE], f32)      # alphas a (3x-replicated over channels)
        th = big.tile([63, PCORE], f32)      # tanh(col/2), later prod2
        cps = big.tile([63, PCORE], f32)     # prefix sums C (triplicated)
        wgt = big.tile([63, PCORE], f32)     # composite weights (triplicated)

        for rep in range(repeats):
            for ti in range(NT):
                sl = slice(ti * TILE, (ti + 1) * TILE)
                q = pq.tile([128, TILE], f32, tag="q")
                nc.tensor.matmul(q[:], coef[:], basis[:, sl], start=True, stop=True)
                nc.scalar.activation(av[:, sl], q[0:63, :], act.Exp)
                nc.scalar.activation(th[:, sl], q[64:127, :], act.Tanh, scale=0.5)
            # a = (a_raw >= 1/255) * min(a_raw, 0.99)   (two frame-wide DVE ops)
            nc.vector.scalar_tensor_tensor(out=av[:], in0=av[:],
                                           scalar=ALPHA_SKIP, in1=av[:],
                                           op0=op.is_ge, op1=op.mult)
            nc.vector.tensor_scalar(out=av[:], in0=av[:],
                                    scalar1=ALPHA_CLAMP, scalar2=None, op0=op.min)
            for ti in range(NT):
                sl = slice(ti * TILE, (ti + 1) * TILE)
                cp = pc_.tile([63, TILE], f32, tag="cp")
                nc.tensor.matmul(cp[:], l3[:, :63], av[0:PHAT, sl], start=True, stop=True)
                nc.scalar.copy(cps[:, sl], cp[:])
            # wgt = (C <= 0.9999) * a * (1 + a - C)     (three frame-wide DVE ops)
            nc.vector.tensor_tensor(out=wgt[:], in0=av[:], in1=cps[:],
                                    op=op.subtract)
            nc.vector.scalar_tensor_tensor(out=wgt[:], in0=wgt[:], scalar=1.0,
                                           in1=av[:], op0=op.add, op1=op.mult)
            nc.vector.scalar_tensor_tensor(out=wgt[:], in0=cps[:], scalar=ACC_BREAK,
                                           in1=wgt[:], op0=op.is_le, op1=op.mult)
            # prod2 = (1 + th) * wgt  -> img = sum 0.5 * prod2 = sum wgt * sigmoid
            nc.vector.scalar_tensor_tensor(out=th[:], in0=th[:],
                                           scalar=1.0, in1=wgt[:], op0=op.add,
                                           op1=op.mult)
            for ti in range(NT):
                sl = slice(ti * TILE, (ti + 1) * TILE)
                g = ti % 4
                if g == 0:
                    img = pi.tile([12, TILE], f32, tag="img")
                nc.tensor.matmul(img[:], zh3[:, g, :], th[:, sl],
                                 start=(g == 0), stop=(g == 3))
                if g == 3:
                    sbimg = work.tile([12, TILE], f32, tag="sbimg")
                    nc.scalar.copy(sbimg[:], img[:])
                    nc.sync.dma_start(out=img_d[ti // 4], in_=sbimg[:])
    nc.compile()
    _NC_CACHE[key] = nc
    return nc


# ---------------------------------------------------------------------------
# Fallback path: original replicated 256-point kernel (any input).
# ---------------------------------------------------------------------------

def _host_baseline(geo):
    f32 = np.float32
    zs = geo["zs"]
    idx = np.arange(N)
    S = ((zs[None, :] < zs[:, None])
         | ((zs[None, :] == zs[:, None]) & (idx[None, :] <= idx[:, None]))).astype(f32)
    Sneg = (np.eye(N, dtype=f32) - S).astype(f32)
    stp = np.zeros((128, 4, 128), f32)
    stn = np.zeros((128, 4, 128), f32)
    for bi in range(2):
        for bj in range(2):
            stp[:, bi * 2 + bj, :] = S[bi * 128:(bi + 1) * 128, bj * 128:(bj + 1) * 128].T
            stn[:, bi * 2 + bj, :] = Sneg[bi * 128:(bi + 1) * 128, bj * 128:(bj + 1) * 128].T
    coefft = np.ascontiguousarray(
        geo["coeffs"].transpose(2, 1, 0).reshape(16, 3 * N)).astype(f32)
    zh = np.zeros((128, 16, 12), f32)
    for g in range(4):
        zh[:, 4 * g + 0, 3 * g:3 * g + 3] = 0.5
        for c in range(3):
            zh[:, 4 * g + 1 + c, 3 * g + c] = 0.5
    return dict(A=geo["A"], stp=stp, stn=stn, coefft=coefft,
                bpix=geo["bpix"], shb=geo["shb"], zh=zh)


def _build_nc_baseline(repeats=1):
    key = ("base", repeats)
    if key in _NC_CACHE:
        return _NC_CACHE[key]
    from contextlib import ExitStack
    import concourse.tile as tile
    from concourse import bacc, mybir

    f32 = mybir.dt.float32
    op = mybir.AluOpType
    act = mybir.ActivationFunctionType

    nc = bacc.Bacc(None, target_bir_lowering=False, debug=False)
    bpix_d = nc.dram_tensor("bpix", [6, PCORE], f32, kind="ExternalInput")
    shb_d = nc.dram_tensor("shb", [16, PCORE], f32, kind="ExternalInput")
    apr_d = nc.dram_tensor("aprime", [6, N], f32, kind="ExternalInput")
    stp_d = nc.dram_tensor("stpos", [128, 4, 128], f32, kind="ExternalInput")
    stn_d = nc.dram_tensor("stneg", [128, 4, 128], f32, kind="ExternalInput")
    cft_d = nc.dram_tensor("coefft", [16, 3 * N], f32, kind="ExternalInput")
    zh_d = nc.dram_tensor("zh", [128, 16, 12], f32, kind="ExternalInput")
    img_d = nc.dram_tensor("img", [NT // 4, 12, TILE], f32, kind="ExternalOutput")

    with tile.TileContext(nc) as tc, ExitStack() as ctx:
        const = ctx.enter_context(tc.tile_pool(name="const", bufs=1))
        work = ctx.enter_context(tc.tile_pool(name="work", bufs=3))
        keep = ctx.enter_context(tc.tile_pool(name="keep", bufs=4))
        ps_q = ctx.enter_context(tc.tile_pool(name="ps_q", bufs=2, space="PSUM"))
        ps_c = ctx.enter_context(tc.tile_pool(name="ps_c", bufs=1, space="PSUM"))
        ps_col = ctx.enter_context(tc.tile_pool(name="ps_col", bufs=2, space="PSUM"))
        ps_img = ctx.enter_context(tc.tile_pool(name="ps_img", bufs=2, space="PSUM"))

        bpix = const.tile([6, PCORE], f32)
        nc.sync.dma_start(out=bpix[:], in_=bpix_d[:])
        shb = const.tile([16, PCORE], f32)
        nc.sync.dma_start(out=shb[:], in_=shb_d[:])
        apr = const.tile([6, N], f32)
        nc.sync.dma_start(out=apr[:], in_=apr_d[:])
        stp = const.tile([128, 4, 128], f32)
        nc.sync.dma_start(out=stp[:], in_=stp_d[:])
        stn = const.tile([128, 4, 128], f32)
        nc.sync.dma_start(out=stn[:], in_=stn_d[:])
        cft = const.tile([16, 3 * N], f32)
        nc.sync.dma_start(out=cft[:], in_=cft_d[:])
        zh = const.tile([128, 16, 12], f32)
        nc.sync.dma_start(out=zh[:], in_=zh_d[:])

        img = None
        for ti_rep in range(NT * repeats):
            ti = ti_rep % NT
            sl = slice(ti * TILE, (ti + 1) * TILE)
            g = ti % 4
            if g == 0:
                img = ps_img.tile([12, TILE], f32, tag="img")
            quads, a_s = [], []
            for b in range(2):
                quad = ps_q.tile([128, TILE], f32, tag="quad")
                nc.tensor.matmul(quad[:], apr[:, b * 128:(b + 1) * 128], bpix[:, sl],
                                 start=True, stop=True)
                t_ = work.tile([128, TILE], f32, tag="t_")
                nc.vector.tensor_scalar(out=t_[:], in0=quad[:], scalar1=LN_CLAMP,
                                        scalar2=None, op0=op.min)
                ex = work.tile([128, TILE], f32, tag="ex")
                nc.scalar.activation(ex[:], t_[:], act.Exp)
                av = keep.tile([128, TILE], f32, tag="av")
                nc.vector.scalar_tensor_tensor(out=av[:], in0=quad[:], scalar=LN_SKIP,
                                               in1=ex[:], op0=op.is_ge, op1=op.mult)
                quads.append(quad)
                a_s.append(av)
            wgts = []
            for b in range(2):
                Cp = ps_c.tile([128, TILE], f32, tag="Cp")
                Cn = ps_c.tile([128, TILE], f32, tag="Cn")
                for bj in range(2):
                    nc.tensor.matmul(Cp[:], stp[:, b * 2 + bj, :], a_s[bj][:],
                                     start=(bj == 0), stop=(bj == 1))
                    nc.tensor.matmul(Cn[:], stn[:, b * 2 + bj, :], a_s[bj][:],
                                     start=(bj == 0), stop=(bj == 1))
                w1 = work.tile([128, TILE], f32, tag="w1")
                nc.vector.scalar_tensor_tensor(out=w1[:], in0=Cn[:], scalar=-1.0,
                                               in1=a_s[b][:], op0=op.subtract, op1=op.mult)
                wgt = keep.tile([128, TILE], f32, tag="wgt")
                nc.vector.scalar_tensor_tensor(out=wgt[:], in0=Cp[:], scalar=ACC_BREAK,
                                               in1=w1[:], op0=op.is_le, op1=op.mult)
                wgts.append(wgt)
            for b in range(2):
                nc.tensor.matmul(img[:], zh[:, 4 * g + 0, :], wgts[b][:],
                                 start=(g == 0 and b == 0), stop=False)
            for c in range(3):
                for b in range(2):
                    col = ps_col.tile([128, TILE], f32, tag="col")
                    nc.tensor.matmul(col[:], cft[:, c * N + b * 128:c * N + (b + 1) * 128],
                                     shb[:, sl], start=True, stop=True)
                    th = work.tile([128, TILE], f32, tag="th")
                    nc.scalar.activation(th[:], col[:], act.Tanh, scale=0.5)
                    prod = work.tile([128, TILE], f32, tag="prod")
                    nc.vector.tensor_mul(prod[:], wgts[b][:], th[:])
                    nc.tensor.matmul(img[:], zh[:, 4 * g + 1 + c, :], prod[:],
                                     start=False, stop=(g == 3 and c == 2 and b == 1))
            if g == 3:
                sbimg = work.tile([12, TILE], f32, tag="sbimg")
                nc.scalar.copy(sbimg[:], img[:])
                nc.sync.dma_start(out=img_d[ti // 4], in_=sbimg[:])
    nc.compile()
    _NC_CACHE[key] = nc
    return nc


def _run(inputs, trace=False, repeats=1):
    from concourse.bass_utils import run_bass_kernel_spmd

    geo = _host_geo(inputs["pointcloud"], inputs["pointcloud_features"],
                    inputs["camera_intrinsics"], inputs["T_camera_pointcloud"])
    sels = _cull_cores(geo)
    if max(len(s) for s in sels) <= PHAT:
        nc = _build_nc_fast(repeats)
        in_maps = _host_fast(geo, sels)
    else:
        nc = _build_nc_baseline(repeats)
        pre = _host_baseline(geo)
        in_maps = []
        for core in range(NCORES):
            p0 = core * PCORE
            in_maps.append({
                "bpix": np.ascontiguousarray(pre["bpix"][:, p0:p0 + PCORE]),
                "shb": np.ascontiguousarray(pre["shb"][:, p0:p0 + PCORE]),
                "aprime": pre["A"],
                "stpos": pre["stp"],
                "stneg": pre["stn"],
                "coefft": pre["coefft"],
                "zh": pre["zh"],
            })
    bkr = run_bass_kernel_spmd(nc, in_maps, list(range(NCORES)), trace=trace)
    out = np.zeros((H, W, 3), np.float32)
    for core in range(NCORES):
        img = bkr.results[core]["img"]  # [NT//4, 12, TILE]
        flat = np.transpose(img.reshape(NT // 4, 4, 3, TILE), (2, 0, 1, 3)).reshape(3, PCORE)
        out[core * ROWS:(core + 1) * ROWS] = flat.reshape(3, ROWS, W).transpose(1, 2, 0)
    return out, bkr


def kernel(**inputs):
    return _run(inputs)[0]
